# revision 40
# baseline (speedup 1.0000x reference)
"""Trainium2 Bass kernel for LongcatFlashMLA prefill (B=2, L=2048, H=16 MLA).

Sharding: core c handles batch c//4 and heads 4*(c%4) .. 4*(c%4)+4 (tensor
parallel over heads, data parallel over batch). Each core computes a partial
output [L, HID] (its heads' contribution through o_proj); host sums the 4
partials per batch.

Causal fast path restructure vs the v1 kernel:
  * The q LoRA chain q_b(rmsnorm(q_a(x))) is algebraically collapsed: the
    per-token rms factor commutes through the second matmul, so the device
    computes x @ (wq_a.T @ wq_b.T) once (6 m-tiles vs 12+6) and applies the
    1/rms(q_a(x)) per-token column scale afterwards.
  * The rms denominators still need the full q_a activations; that pass and
    the kv_a latent projection are FEATURE-SPLIT across the 4 cores of a
    batch group (each core computes 384 of 1536 q_a rows and 128 of 512
    latent rows). A 16KB AllReduce combines the sum-of-squares partials and
    a 2MB AllGather rebuilds the full latent, both overlapped behind the
    combined-q matmuls.
  * Scores are computed pre-transposed (S^T[kpos, q]) so the exp output is
    directly the attn@V operand: no per-block PE transpose / DVE copy.
    Softmax denominators come from ones-vector matmuls accumulated alongside
    attn@V in the same PSUM bank; the 1/rowsum scale is applied at the tiny
    [VD, q] output eviction as before.

Device-side layout is feature-major ("T" = [feature, seq]) so every matmul
operand is already contraction-major; all transposes are done on host, where
they are free. LN weights and all scalar factors (SQ, SKV, SCALE) are folded
into adjacent weight matrices on host. Matmuls run in bf16 with fp32 PSUM
accumulation. Softmax skips max-subtraction (scores from 0.02-scale weights
are bounded well inside exp's fp32 range).
"""

import math
import sys
from contextlib import ExitStack

import numpy as np

if "/opt/trn_rl_repo" not in sys.path:
    sys.path.insert(0, "/opt/trn_rl_repo")

import ml_dtypes

B, L, HID = 2, 2048, 2048
H = 16
QL = 1536
KVL = 512
ROPE = 64
NOPE = 128
VD = 128
QKD = NOPE + ROPE
SCALE = QKD ** -0.5
EPS = 1e-5
THETA = 10000.0
SQ = (HID / QL) ** 0.5
SKV = (HID / KVL) ** 0.5

NCORES = 8
GPB = 4              # head-groups (cores) per batch
HPG = H // GPB       # heads per core
CH = 512             # L chunk processed per pipeline stage
NCH = L // CH
QT = CH // 128       # q-tiles (128 rows) per chunk
KT = HID // 128      # k-tiles of the HID contraction
MT = QL // 128       # k-tiles of the QL contraction
QBT = (HPG * QKD) // 128   # m-tiles of q output (768 rows -> 6)
KVT = KVL // 128     # 4
NBLK = L // 128      # 16 kpos blocks total
QAS = QL // GPB      # feature-split q_a rows per core (384)
QAST = QAS // 128    # m-tiles of the q_a split (3)

GROUPS = [[0, 1, 2, 3], [4, 5, 6, 7]]

BF16 = ml_dtypes.bfloat16

_PROG_CACHE = {}


def _bf(a):
    return np.ascontiguousarray(a.astype(np.float32)).astype(BF16)


def _emit_fast(ctx, tc, t):
    """Causal fast path (feature-split projections + collectives + S^T)."""
    import concourse.bass as bass
    from concourse import mybir

    nc = tc.nc
    f32 = mybir.dt.float32
    bf = mybir.dt.bfloat16
    AF = mybir.ActivationFunctionType
    MULT = mybir.AluOpType.mult

    const = ctx.enter_context(tc.tile_pool(name="const", bufs=1))
    wstream = ctx.enter_context(tc.tile_pool(name="wstream", bufs=2))
    xt_p = ctx.enter_context(tc.tile_pool(name="xt", bufs=2))
    sq_p = ctx.enter_context(tc.tile_pool(name="sq", bufs=2))
    ev_p = ctx.enter_context(tc.tile_pool(name="ev", bufs=2))
    qs_p = ctx.enter_context(tc.tile_pool(name="qs", bufs=4))
    rp_p = ctx.enter_context(tc.tile_pool(name="rp", bufs=4))
    rope_p = ctx.enter_context(tc.tile_pool(name="rope", bufs=2))
    rows_p = ctx.enter_context(tc.tile_pool(name="rows", bufs=1))
    persist = ctx.enter_context(tc.tile_pool(name="persist", bufs=1))
    P_p = ctx.enter_context(tc.tile_pool(name="P", bufs=2))
    oh_p = ctx.enter_context(tc.tile_pool(name="oh", bufs=2))
    den_p = ctx.enter_context(tc.tile_pool(name="den", bufs=1))
    st_p = ctx.enter_context(tc.tile_pool(name="stage", bufs=2))

    psA = ctx.enter_context(tc.tile_pool(name="psA", bufs=2, space="PSUM"))
    psS = ctx.enter_context(tc.tile_pool(name="psS", bufs=2, space="PSUM"))
    psV = ctx.enter_context(tc.tile_pool(name="psV", bufs=2, space="PSUM"))
    psW = ctx.enter_context(tc.tile_pool(name="psW", bufs=2, space="PSUM"))

    dram = ctx.enter_context(tc.tile_pool(name="dram", bufs=1, space="DRAM"))

    # ---- phase-1-critical constants first (DMA queue is serial)
    wkva_t = const.tile([128, KT, 128 + ROPE], bf, name="wkva")
    nc.sync.dma_start(out=wkva_t[:], in_=t["wkva"][:])
    cos_t = const.tile([128, L], bf, name="cos")
    sin_t = const.tile([128, L], bf, name="sin")
    perm_t = const.tile([128, 128], bf, name="perm")
    dup_t = const.tile([64, 128], bf, name="dup")
    triT_t = const.tile([128, 128], f32, name="triT")
    onec_t = const.tile([128, 1], bf, name="onec")
    nc.vector.memset(onec_t[:], 1.0)
    oner32_t = const.tile([1, 128], f32, name="oner32")
    nc.vector.memset(oner32_t[:], 1.0)
    zero_t = const.tile([128, 1], f32, name="zero")
    nc.vector.memset(zero_t[:], 0.0)
    eps_t = const.tile([128, 1], f32, name="eps")
    nc.vector.memset(eps_t[:], EPS)

    # ---- persistent tiles
    latk = [persist.tile([128, L], bf, name=f"latk{k}") for k in range(KVT)]
    kno_t = [persist.tile([128, L], bf, name=f"kno{h}") for h in range(HPG)]
    kpe_t = persist.tile([128, L], bf, name="kpe")
    v_t = [persist.tile([128, NBLK, 2 * VD], bf, name=f"v{g}") for g in range(2)]
    r_q = rows_p.tile([1, L], f32, name="rq")
    r_kv = rows_p.tile([1, L], f32, name="rkv")

    # ---- collective DRAM buffers (latent gathered in two halves so the
    # sum-of-squares AllReduce can run between them on the collective cores)
    HL = L // 2
    gin_h = [dram.tile([128, HL], bf, name=f"gin{i}") for i in range(2)]
    gout_h = [dram.tile([GPB, 128, HL], bf, name=f"gout{i}") for i in range(2)]
    rin = dram.tile([2, L], f32, name="rin")
    rout = dram.tile([2, L], f32, name="rout")

    def rope_apply(dst, src, pp, cs):
        tmp = rope_p.tile([128, CH], bf, name="rtmp")
        nc.vector.tensor_tensor(tmp[:], pp[:], sin_t[:, cs], op=MULT)
        nc.vector.tensor_tensor(dst[:], src[:], cos_t[:, cs], op=MULT)
        nc.vector.tensor_add(dst[:], dst[:], tmp[:])

    # ================= phase 1: feature-split projections ================
    # latq(c0) and latq(c1) run first so the first latent-half AllGather is
    # on the collective cores by ~20us; the q_a sum-of-squares AllReduce is
    # ordered LAST (its result is needed latest). The shared-key
    # dup/perm/rope chain of chunk c is deferred so its Act-engine
    # evictions never stall the PE.
    ck_pend = {}
    dup_pend = {}

    def kpe_rope(c):
        pd = psA.tile([128, CH], f32, name="pa")
        nc.tensor.matmul(pd[:], dup_t[:], ck_pend.pop(c)[:], start=True,
                         stop=True)
        dup_sb = rope_p.tile([128, CH], bf, name="rdup")
        nc.scalar.copy(dup_sb[:], pd[:])
        return dup_sb

    def kpe_rope2(c, dup_sb):
        cs = slice(c * CH, (c + 1) * CH)
        pp = psA.tile([128, CH], f32, name="pa")
        nc.tensor.matmul(pp[:], perm_t[:], dup_sb[:], start=True, stop=True)
        rope_apply(kpe_t[:, cs], dup_sb, pp, cs)

    def load_xt(c, hi=True):
        xt = xt_p.tile([128, KT, CH], bf, name="xt")
        if hi:
            with tc.high_priority():
                nc.sync.dma_start(out=xt[:], in_=t["xT"][c])
        else:
            nc.sync.dma_start(out=xt[:], in_=t["xT"][c])
        return xt

    def latq_block(c, xt):
        cs = slice(c * CH, (c + 1) * CH)
        pa = psA.tile([128, CH], f32, name="pa")
        for k in range(KT):
            nc.tensor.matmul(pa[:], wkva_t[:, k, :128], xt[:, k, :],
                             start=(k == 0), stop=(k == KT - 1))
        lat_sb = ev_p.tile([128, CH], bf, name="lat")
        with tc.high_priority():
            nc.scalar.copy(lat_sb[:], pa[:])
            hcs = slice((c % 2) * CH, (c % 2) * CH + CH)
            nc.gpsimd.dma_start(out=gin_h[c // 2][:, hcs], in_=lat_sb[:])
        sql = sq_p.tile([128, CH], bf, name="sq")
        nc.scalar.square(sql[:], pa[:])
        pr = psW.tile([128, CH], f32, name="pw")
        nc.tensor.matmul(pr[:1, :], onec_t[:], sql[:], start=True, stop=True)
        with tc.high_priority():
            nc.scalar.copy(r_kv[:, cs], pr[:1, :])

    def qa_block(c, xt):
        cs = slice(c * CH, (c + 1) * CH)
        sqs = []
        for m in range(QAST):
            wcol = wstream.tile([128, KT, 128], bf, name="wcol")
            nc.sync.dma_start(out=wcol[:], in_=t["wqa"][m])
            pa = psA.tile([128, CH], f32, name="pa")
            for k in range(KT):
                nc.tensor.matmul(pa[:], wcol[:, k, :],
                                 xt[:, k, :], start=(k == 0),
                                 stop=(k == KT - 1))
            sq = sq_p.tile([128, CH], bf, name="sq")
            nc.scalar.square(sq[:], pa[:])
            sqs.append(sq)
        pr = psW.tile([128, CH], f32, name="pw")
        for m, sq in enumerate(sqs):
            nc.tensor.matmul(pr[:1, :], onec_t[:], sq[:],
                             start=(m == 0), stop=(m == len(sqs) - 1))
        with tc.high_priority():
            nc.scalar.copy(r_q[:, cs], pr[:1, :])

    def kpe_block(c, xt):
        pa = psA.tile([128, CH], f32, name="pa")
        for k in range(KT):
            nc.tensor.matmul(pa[:ROPE, :], wkva_t[:, k, 128:128 + ROPE],
                             xt[:, k, :], start=(k == 0), stop=(k == KT - 1))
        ck_sb = ev_p.tile([64, CH], bf, name="ck")
        nc.scalar.copy(ck_sb[:], pa[:ROPE, :])
        ck_pend[c] = ck_sb

    xt0 = load_xt(0)
    xt1 = load_xt(1)
    latq_block(0, xt0)
    latq_block(1, xt1)
    with tc.high_priority():
        nc.gpsimd.collective_compute(
            "AllGather", mybir.AluOpType.bypass, replica_groups=GROUPS,
            ins=[gin_h[0][:]], outs=[gout_h[0][:]])
    qa_block(0, xt0)
    nc.sync.dma_start(out=cos_t[:], in_=t["cos"][:])
    nc.sync.dma_start(out=sin_t[:], in_=t["sin"][:])
    nc.sync.dma_start(out=perm_t[:], in_=t["perm"][:])
    nc.sync.dma_start(out=dup_t[:], in_=t["dup"][:])
    nc.sync.dma_start(out=triT_t[:], in_=t["triT"][:])
    kpe_block(0, xt0)
    xt2 = load_xt(2)
    qa_block(1, xt1)
    dup_pend[0] = kpe_rope(0)
    latq_block(2, xt2)
    kpe_block(1, xt1)
    xt3 = load_xt(3)
    kpe_rope2(0, dup_pend.pop(0))
    latq_block(3, xt3)
    with tc.high_priority():
        nc.gpsimd.collective_compute(
            "AllGather", mybir.AluOpType.bypass, replica_groups=GROUPS,
            ins=[gin_h[1][:]], outs=[gout_h[1][:]])
    qa_block(2, xt2)
    dup_pend[1] = kpe_rope(1)
    kpe_block(2, xt2)
    kpe_rope2(1, dup_pend.pop(1))
    qa_block(3, xt3)
    dup_pend[2] = kpe_rope(2)
    kpe_block(3, xt3)
    kpe_rope2(2, dup_pend.pop(2))

    with tc.high_priority():
        nc.gpsimd.dma_start(out=rin[0:1, :], in_=r_q[:])
        nc.gpsimd.dma_start(out=rin[1:2, :], in_=r_kv[:])
        nc.gpsimd.collective_compute(
            "AllReduce", mybir.AluOpType.add, replica_groups=GROUPS,
            ins=[rin[:]], outs=[rout[:]])

    # ================= phase 2: combined q projection (per chunk) ========
    qs_c = {}
    rp_c = {}
    for c in range(NCH):
        cs = slice(c * CH, (c + 1) * CH)
        xt = xt_p.tile([128, KT, CH], bf, name="xt")
        nc.sync.dma_start(out=xt[:], in_=t["xT"][c])
        qs = qs_p.tile([128, QBT, CH], bf, name="qs")
        for m in range(QBT):
            wcol = wstream.tile([128, KT, 128], bf, name="wcol")
            nc.sync.dma_start(out=wcol[:], in_=t["wcomb"][m])
            pa = psA.tile([128, CH], f32, name="pa")
            for k in range(KT):
                nc.tensor.matmul(pa[:], wcol[:, k, :],
                                 xt[:, k, :], start=(k == 0),
                                 stop=(k == KT - 1))
            nc.scalar.copy(qs[:, m, :], pa[:])
        if c - 1 in ck_pend:
            dup_pend[c - 1] = kpe_rope(c - 1)
        # rope the two pe pair-tiles (scale-free: rms factor applied after)
        rp = rp_p.tile([128, 2, CH], bf, name="rp")
        for pair in range(2):
            pp = psA.tile([128, CH], f32, name="pa")
            nc.tensor.matmul(pp[:], perm_t[:], qs[:, HPG + pair, :],
                             start=True, stop=True)
            rope_apply(rp[:, pair, :], qs[:, HPG + pair, :], pp, cs)
        if c - 1 in dup_pend:
            kpe_rope2(c - 1, dup_pend.pop(c - 1))
        qs_c[c], rp_c[c] = qs, rp
    kpe_rope2(NCH - 1, kpe_rope(NCH - 1))

    # remaining constants (needed from phase 4 on; keep them off the front
    # of the serial DMA queue)
    wemb_t = const.tile([128, KVT, HPG * NOPE], bf, name="wemb")
    nc.sync.dma_start(out=wemb_t[:], in_=t["wemb"][:])
    wunb_t = const.tile([128, KVT, HPG * VD], bf, name="wunb")
    nc.sync.dma_start(out=wunb_t[:], in_=t["wunb"][:])
    wot_t = const.tile([128, HPG, HID], bf, name="wot")
    nc.sync.dma_start(out=wot_t[:], in_=t["wot"][:])

    # ================= phase 3: rms factors + latent gather/scale ========
    with tc.high_priority():
        nc.gpsimd.dma_start(out=r_q[:], in_=rout[0:1, :])
        nc.gpsimd.dma_start(out=r_kv[:], in_=rout[1:2, :])
        nc.scalar.activation(r_kv[:], r_kv[:], AF.Sqrt,
                             bias=eps_t[:1, :], scale=1.0 / KVL)
        nc.vector.reciprocal(r_kv[:], r_kv[:])
        nc.scalar.activation(r_q[:], r_q[:], AF.Sqrt,
                             bias=eps_t[:1, :], scale=1.0 / QL)
        nc.vector.reciprocal(r_q[:], r_q[:])

    def lat_half(hb):
        for k in range(KVT):
            nc.gpsimd.dma_start(out=latk[k][:, hb * HL:(hb + 1) * HL],
                                in_=gout_h[hb][k])
        for c in (2 * hb, 2 * hb + 1):
            cs = slice(c * CH, (c + 1) * CH)
            pb = psA.tile([128, CH], f32, name="pa")
            nc.tensor.matmul(pb[:], oner32_t[:], r_kv[:, cs], start=True,
                             stop=True)
            for k in range(KVT):
                nc.vector.tensor_tensor(latk[k][:, cs], latk[k][:, cs],
                                        pb[:], op=MULT)

    def embed_half(hb):
        for h in range(HPG):
            for grp in (2 * hb, 2 * hb + 1):
                gs = slice(grp * CH, (grp + 1) * CH)
                pa = psA.tile([128, CH], f32, name="pa")
                for k in range(KVT):
                    nc.tensor.matmul(pa[:],
                                     wemb_t[:, k, h * NOPE:(h + 1) * NOPE],
                                     latk[k][:, gs], start=(k == 0),
                                     stop=(k == KVT - 1))
                nc.scalar.copy(kno_t[h][:, gs], pa[:])
        for g in range(2):
            for bt in range(hb * 8, hb * 8 + 8):
                bs = slice(bt * 128, (bt + 1) * 128)
                pa = psA.tile([128, CH], f32, name="pa")
                for k in range(KVT):
                    nc.tensor.matmul(
                        pa[:, :2 * VD], latk[k][:, bs],
                        wunb_t[:, k, g * 2 * VD:(g + 1) * 2 * VD],
                        start=(k == 0), stop=(k == KVT - 1))
                nc.scalar.copy(v_t[g][:, bt, :], pa[:, :2 * VD])

    lat_half(0)
    embed_half(0)
    lat_half(1)

    for c in range(NCH):
        # per-token q rms factor applied to nope tiles and roped pe tiles
        cs = slice(c * CH, (c + 1) * CH)
        qs, rp = qs_c[c], rp_c[c]
        pb = psA.tile([128, CH], f32, name="pa")
        nc.tensor.matmul(pb[:], oner32_t[:], r_q[:, cs], start=True,
                         stop=True)
        for m in range(HPG):
            nc.vector.tensor_tensor(qs[:, m, :], qs[:, m, :], pb[:], op=MULT)
        for pair in range(2):
            nc.vector.tensor_tensor(rp[:, pair, :], rp[:, pair, :], pb[:],
                                    op=MULT)

    embed_half(1)

    # ================= phase 5/6: attention (S^T) + o_proj ===============
    # Pipelined at (qt, h, kpos-group) granularity: scores/exp of item i+2
    # are emitted before attn@V of item i, so the PE never waits on the
    # Act engine; denominator finalization trails one more step.
    GRP = 4  # kpos blocks per psum tile

    def s_group(qt, h, ci):
        c = qt // QT
        qs, rp = qs_c[c], rp_c[c]
        ql = qt % QT
        qsl = slice(ql * 128, (ql + 1) * 128)
        hb = (h % 2) * 64
        nb = qt + 1
        b0 = ci * GRP
        nbt = min(GRP, nb - b0)
        ps = psS.tile([128, CH], f32, name="ps")
        for j in range(nbt):
            bt = b0 + j
            ksl = slice(bt * 128, (bt + 1) * 128)
            jsl = slice(j * 128, (j + 1) * 128)
            nc.tensor.matmul(ps[:, jsl], kno_t[h][:, ksl], qs[:, h, qsl],
                             start=True, stop=False)
            nc.tensor.matmul(ps[:, jsl], kpe_t[hb:hb + 64, ksl],
                             rp[hb:hb + 64, h // 2, qsl],
                             start=False, stop=True)
            if bt == qt:
                nc.vector.tensor_add(ps[:, jsl], ps[:, jsl], triT_t[:])
        w = nbt * 128
        P_sb = P_p.tile([128, CH], bf, name="P")
        nc.scalar.activation(P_sb[:, :w], ps[:, :w], AF.Exp, bias=zero_t[:])
        return P_sb

    pv_cur = [None]

    def av_group(qt, h, ci, P_sb):
        nb = qt + 1
        b0 = ci * GRP
        nbt = min(GRP, nb - b0)
        nsc = (nb + GRP - 1) // GRP
        if ci == 0:
            pv_cur[0] = psV.tile([128, CH], f32, name="pv")
        pv = pv_cur[0]
        for j in range(nbt):
            bt = b0 + j
            blk = P_sb[:, j * 128:(j + 1) * 128]
            first = (ci == 0 and j == 0)
            last = (ci == nsc - 1 and j == nbt - 1)
            nc.tensor.matmul(
                pv[:, :VD],
                v_t[h // 2][:, bt, (h % 2) * VD:(h % 2 + 1) * VD],
                blk, start=first, stop=last)
            nc.tensor.matmul(pv[:1, VD:VD + 128], onec_t[:], blk,
                             start=first, stop=last)
        return pv

    def fin_block(h, pv, oh):
        den = den_p.tile([1, 128], f32, name="den")
        nc.vector.tensor_copy(den[:], pv[:1, VD:VD + 128])
        rec = den_p.tile([1, 128], f32, name="rec")
        nc.vector.reciprocal(rec[:], den[:])
        pb = psA.tile([128, CH], f32, name="pa")
        nc.tensor.matmul(pb[:, :128], oner32_t[:], rec[:],
                         start=True, stop=True)
        bcs = den_p.tile([128, 128], f32, name="bcs")
        nc.vector.tensor_copy(bcs[:], pb[:, :128])
        nc.vector.tensor_tensor(oh[:, h, :], pv[:, :VD], bcs[:], op=MULT)

    def o_proj(qt, oh):
        for nn in range(HID // 512):
            pw = psW.tile([128, 512], f32, name="pw")
            for h in range(HPG):
                nc.tensor.matmul(pw[:], oh[:, h, :],
                                 wot_t[:, h, nn * 512:(nn + 1) * 512],
                                 start=(h == 0), stop=(h == HPG - 1))
            stg = st_p.tile([128, 512], bf, name="stg")
            nc.vector.tensor_copy(stg[:], pw[:])
            nc.sync.dma_start(
                out=t["out"][qt * 128:(qt + 1) * 128,
                             nn * 512:(nn + 1) * 512],
                in_=stg[:])

    flat = []
    for qt in range(NBLK):
        nsc = (qt + 1 + GRP - 1) // GRP
        for h in range(HPG):
            for ci in range(nsc):
                flat.append((qt, h, ci))
    LEAD = 2
    P_buf = {}
    oh_tiles = {}
    fins = []
    oprojs = []
    for i in range(len(flat) + LEAD + 2):
        # 1. attn@V for item i - LEAD (frees the P buffer item i will reuse)
        j = i - LEAD
        if 0 <= j < len(flat):
            qt, h, ci = flat[j]
            pv = av_group(qt, h, ci, P_buf.pop(j))
            if ci == (qt + 1 + GRP - 1) // GRP - 1:
                fins.append((qt, h, pv))
        # 2. scores/exp for item i (leads the AV stream by LEAD steps)
        if i < len(flat):
            qt, h, ci = flat[i]
            if h == 0 and ci == 0:
                oh_tiles[qt] = oh_p.tile([128, HPG, VD], bf, name="oh")
            P_buf[i] = s_group(qt, h, ci)
        # 3. one finalization queued in an earlier step (its Act/DVE
        #    dependencies have had a full step to complete)
        if fins:
            qtf, hf, pvf = fins.pop(0)
            fin_block(hf, pvf, oh_tiles[qtf])
            if hf == HPG - 1:
                oprojs.append(qtf)
        # 4. one o_proj whose finalization was emitted in an earlier step
        if len(oprojs) > (1 if i < len(flat) else 0):
            qto = oprojs.pop(0)
            o_proj(qto, oh_tiles.pop(qto))
    while oprojs:
        qto = oprojs.pop(0)
        o_proj(qto, oh_tiles.pop(qto))


def _emit_full(ctx, tc, t):
    """Non-causal (all-true mask) fallback: v1 kernel structure."""
    import concourse.bass as bass
    from concourse import mybir

    nc = tc.nc
    f32 = mybir.dt.float32
    bf = mybir.dt.bfloat16
    AF = mybir.ActivationFunctionType
    MULT = mybir.AluOpType.mult
    X = mybir.AxisListType.X

    const = ctx.enter_context(tc.tile_pool(name="const", bufs=1))
    wstream = ctx.enter_context(tc.tile_pool(name="wstream", bufs=3))
    xt_p = ctx.enter_context(tc.tile_pool(name="xt", bufs=1))
    qa_p = ctx.enter_context(tc.tile_pool(name="qa", bufs=1))
    ckv_p = ctx.enter_context(tc.tile_pool(name="ckv", bufs=1))
    sq_p = ctx.enter_context(tc.tile_pool(name="sq", bufs=14))
    qs_p = ctx.enter_context(tc.tile_pool(name="qs", bufs=1))
    rope_p = ctx.enter_context(tc.tile_pool(name="rope", bufs=2))
    persist = ctx.enter_context(tc.tile_pool(name="persist", bufs=1))
    attn_p = ctx.enter_context(tc.tile_pool(name="attn", bufs=2))
    pt_p = ctx.enter_context(tc.tile_pool(name="pt", bufs=3))
    oh_p = ctx.enter_context(tc.tile_pool(name="oh", bufs=2))
    st_p = ctx.enter_context(tc.tile_pool(name="stage", bufs=2))
    sm_p = ctx.enter_context(tc.tile_pool(name="sm", bufs=3))

    psA = ctx.enter_context(tc.tile_pool(name="psA", bufs=2, space="PSUM"))
    psS = ctx.enter_context(tc.tile_pool(name="psS", bufs=2, space="PSUM"))
    psT = ctx.enter_context(tc.tile_pool(name="psT", bufs=2, space="PSUM"))
    psVW = ctx.enter_context(tc.tile_pool(name="psVW", bufs=2, space="PSUM"))

    wkva_t = const.tile([128, KT, KVL + ROPE], bf, name="wkva")
    nc.sync.dma_start(out=wkva_t[:], in_=t["wkva"].rearrange("k p n -> p k n"))
    wemb_t = const.tile([128, KVT, HPG * NOPE], bf, name="wemb")
    nc.sync.dma_start(out=wemb_t[:], in_=t["wemb"].rearrange("k p n -> p k n"))
    wunb_t = const.tile([128, KVT, HPG * VD], bf, name="wunb")
    nc.sync.dma_start(out=wunb_t[:], in_=t["wunb"].rearrange("k p n -> p k n"))
    wot_t = const.tile([128, HPG, HID], bf, name="wot")
    nc.sync.dma_start(out=wot_t[:], in_=t["wot"].rearrange("h p n -> p h n"))
    cos_t = const.tile([128, L], bf, name="cos")
    nc.sync.dma_start(out=cos_t[:], in_=t["cos"][:])
    sin_t = const.tile([128, L], bf, name="sin")
    nc.sync.dma_start(out=sin_t[:], in_=t["sin"][:])
    perm_t = const.tile([128, 128], bf, name="perm")
    nc.sync.dma_start(out=perm_t[:], in_=t["perm"][:])
    dup_t = const.tile([64, 128], bf, name="dup")
    nc.sync.dma_start(out=dup_t[:], in_=t["dup"][:])
    id_t = const.tile([128, 128], bf, name="ident")
    nc.sync.dma_start(out=id_t[:], in_=t["ident"][:])
    id32_t = const.tile([128, 128], f32, name="ident32")
    nc.sync.dma_start(out=id32_t[:], in_=t["ident32"][:])
    onec_t = const.tile([128, 1], bf, name="onec")
    nc.vector.memset(onec_t[:], 1.0)
    oner_t = const.tile([1, 128], bf, name="oner")
    nc.vector.memset(oner_t[:], 1.0)
    zero_t = const.tile([128, 1], f32, name="zero")
    nc.vector.memset(zero_t[:], 0.0)
    eps_t = const.tile([128, 1], f32, name="eps")
    nc.vector.memset(eps_t[:], EPS)

    kno_t = [persist.tile([128, L], bf, name=f"kno{h}") for h in range(HPG)]
    kpe_t = persist.tile([128, L], bf, name="kpe")
    v_t = [persist.tile([128, NBLK, 2 * VD], bf, name=f"v{g}") for g in range(2)]

    rq_p = ctx.enter_context(tc.tile_pool(name="rq", bufs=(NCH + 1) * QT))

    def proj_norm(xt, wget, mt, rows_list, out_tile, n_sq, inv_n, cs,
                  apply=True):
        sqs = []
        for m in range(mt):
            rows = rows_list[m]
            lhs_src = wget(m)
            pa = psA.tile([128, CH], f32, name="pa")
            for k in range(KT):
                nc.tensor.matmul(
                    pa[:rows, :], lhs_src(k, rows), xt[:, k, :],
                    start=(k == 0), stop=(k == KT - 1),
                )
            nc.scalar.copy(out_tile[:rows, m, :], pa[:rows, :])
            if m < n_sq:
                sq = sq_p.tile([128, CH], bf, name="sq")
                nc.scalar.square(sq[:], pa[:])
                sqs.append(sq)
        pss = psA.tile([128, CH], f32, name="pa")
        for m, sq in enumerate(sqs):
            nc.tensor.matmul(
                pss[:1, :], onec_t[:], sq[:],
                start=(m == 0), stop=(m == len(sqs) - 1),
            )
        rs = sm_p.tile([1, CH], f32, name="rs")
        nc.scalar.activation(rs[:], pss[:1, :], AF.Sqrt, bias=eps_t[:1, :],
                             scale=inv_n)
        rr = sm_p.tile([1, CH], f32, name="rr")
        nc.vector.reciprocal(rr[:], rs[:])
        if apply:
            rb = sm_p.tile([1, CH], bf, name="rb")
            nc.vector.tensor_copy(rb[:], rr[:])
            rbp = psA.tile([128, CH], f32, name="pa")
            nc.tensor.matmul(rbp[:], oner_t[:], rb[:], start=True, stop=True)
            for m in range(n_sq):
                nc.vector.tensor_tensor(out_tile[:, m, :], out_tile[:, m, :],
                                        rbp[:], op=MULT)
            return None
        rqs = []
        for q in range(QT):
            rtp = psVW.tile([128, 512], f32, name="pvw")
            nc.tensor.transpose(rtp[:, :1], rr[:1, q * 128:(q + 1) * 128],
                                id32_t[:1, :1])
            rq = rq_p.tile([128, 1], f32, name="rqc")
            nc.vector.tensor_copy(rq[:], rtp[:, :1])
            rqs.append(rq)
        return rqs

    def rope_apply(dst, src, pp, rows, cs):
        tmp = rope_p.tile([128, CH], bf, name="rtmp")
        nc.vector.tensor_tensor(tmp[:rows, :], pp[:rows, :],
                                sin_t[:rows, cs], op=MULT)
        nc.vector.tensor_tensor(dst[:rows, :], src[:rows, :],
                                cos_t[:rows, cs], op=MULT)
        nc.vector.tensor_add(dst[:rows, :], dst[:rows, :], tmp[:rows, :])

    def phases_abc(c):
        cs = slice(c * CH, (c + 1) * CH)
        xt = xt_p.tile([128, KT, CH], bf, name="xt")
        nc.sync.dma_start(out=xt[:], in_=t["xT"][c].rearrange("k p n -> p k n"))

        qa = qa_p.tile([128, MT, CH], bf, name="qa")

        def wqa_get(m):
            wcol = wstream.tile([128, KT, 128], bf, name="wqa")
            nc.sync.dma_start(out=wcol[:],
                              in_=t["wqa"][m].rearrange("k p j -> p k j"))
            return lambda k, rows: wcol[:, k, :rows]

        rqs = proj_norm(xt, wqa_get, MT, [128] * MT, qa, MT, 1.0 / QL, cs,
                        apply=False)

        ckv = ckv_p.tile([128, KVT + 1, CH], bf, name="ckv")

        def wkva_get(m):
            return lambda k, rows: wkva_t[:, k, m * 128: m * 128 + rows]

        proj_norm(xt, wkva_get, KVT + 1, [128] * KVT + [ROPE], ckv, KVT,
                  1.0 / KVL, cs)

        qs = qs_p.tile([128, QBT, CH], bf, name="qs")
        for m in range(QBT):
            wcol = wstream.tile([128, MT, 128], bf, name="wqb")
            nc.sync.dma_start(out=wcol[:],
                              in_=t["wqb"][m].rearrange("k p j -> p k j"))
            pa = psA.tile([128, CH], f32, name="pa")
            for k in range(MT):
                nc.tensor.matmul(pa[:], wcol[:, k, :], qa[:, k, :],
                                 start=(k == 0), stop=(k == MT - 1))
            nc.scalar.copy(qs[:, m, :], pa[:])

        rp = rope_p.tile([128, 2, CH], bf, name="roped")
        for pair in range(2):
            pp = psA.tile([128, CH], f32, name="pa")
            nc.tensor.matmul(pp[:], perm_t[:], qs[:, HPG + pair, :],
                             start=True, stop=True)
            rope_apply(rp[:, pair, :], qs[:, HPG + pair, :], pp, 128, cs)
        pd = psA.tile([128, CH], f32, name="pa")
        nc.tensor.matmul(pd[:], dup_t[:], ckv[:ROPE, KVT, :],
                         start=True, stop=True)
        dup_sb = rope_p.tile([128, CH], bf, name="rdup")
        nc.scalar.copy(dup_sb[:], pd[:])
        pp = psA.tile([128, CH], f32, name="pa")
        nc.tensor.matmul(pp[:], perm_t[:], dup_sb[:], start=True, stop=True)
        rope_apply(kpe_t[:, cs], dup_sb, pp, 128, cs)

        for h in range(HPG):
            pa = psA.tile([128, CH], f32, name="pa")
            for k in range(KVT):
                nc.tensor.matmul(pa[:], wemb_t[:, k, h * NOPE:(h + 1) * NOPE],
                                 ckv[:, k, :], start=(k == 0),
                                 stop=(k == KVT - 1))
            nc.scalar.copy(kno_t[h][:, cs], pa[:])
        for g in range(2):
            for pi in range(QT):
                pa = psA.tile([128, CH], f32, name="pa")
                for k in range(KVT):
                    nc.tensor.matmul(
                        pa[:, :2 * VD],
                        ckv[:, k, pi * 128:(pi + 1) * 128],
                        wunb_t[:, k, g * 2 * VD:(g + 1) * 2 * VD],
                        start=(k == 0), stop=(k == KVT - 1),
                    )
                nc.scalar.copy(v_t[g][:, c * QT + pi, :], pa[:, :2 * VD])
        return qs, rp, rqs

    def attention(c, qs, rp, rqs):
        for ql in range(QT):
            qt = c * QT + ql
            nb = NBLK
            wtot = nb * 128
            nsc = (wtot + 511) // 512
            qsl = slice(ql * 128, (ql + 1) * 128)
            oh = oh_p.tile([128, HPG, VD], bf, name="oh")
            for h in range(HPG):
                lhs_nope = qs[:, h, qsl]
                lhs_pe = rp[(h % 2) * 64:(h % 2) * 64 + 64, h // 2, qsl]
                P_sb = attn_p.tile([128, L], bf, name="P")
                sums = sm_p.tile([128, 4], f32, name="sums")
                for ci in range(nsc):
                    w = min(512, wtot - ci * 512)
                    ksl = slice(ci * 512, ci * 512 + w)
                    ps = psS.tile([128, 512], f32, name="ps")
                    nc.tensor.matmul(ps[:, :w], lhs_nope, kno_t[h][:, ksl],
                                     start=True, stop=False)
                    hb = (h % 2) * 64
                    nc.tensor.matmul(ps[:, :w], lhs_pe,
                                     kpe_t[hb:hb + 64, ksl],
                                     start=False, stop=True)
                    nc.scalar.activation(P_sb[:, ci * 512:ci * 512 + w],
                                         ps[:, :w], AF.Exp,
                                         bias=zero_t[:], scale=rqs[ql][:],
                                         accum_out=sums[:, ci:ci + 1])
                tot = sm_p.tile([128, 1], f32, name="tot")
                nc.vector.reduce_sum(tot[:], sums[:, :nsc], axis=X)
                rec = sm_p.tile([128, 1], f32, name="rec")
                nc.vector.reciprocal(rec[:], tot[:])
                rt = psVW.tile([128, 512], f32, name="pvw")
                nc.tensor.transpose(rt[:1, :128], rec[:], id32_t[:])
                rts = sm_p.tile([1, 128], bf, name="rts")
                nc.scalar.copy(rts[:], rt[:1, :128])
                bc = psVW.tile([128, 512], f32, name="pvw")
                nc.tensor.matmul(bc[:, :128], oner_t[:], rts[:],
                                 start=True, stop=True)
                bcs = sm_p.tile([128, 128], f32, name="bcs")
                nc.scalar.copy(bcs[:], bc[:, :128])
                pv = psVW.tile([128, 512], f32, name="pvw")
                for bt in range(nb):
                    ptp = psT.tile([128, 128], bf, name="pt")
                    nc.tensor.transpose(ptp[:], P_sb[:, bt * 128:(bt + 1) * 128],
                                        id_t[:])
                    ptb = pt_p.tile([128, 128], bf, name="ptb")
                    nc.vector.tensor_copy(ptb[:], ptp[:])
                    nc.tensor.matmul(
                        pv[:, :VD],
                        v_t[h // 2][:, bt, (h % 2) * VD:(h % 2 + 1) * VD],
                        ptb[:], start=(bt == 0), stop=(bt == nb - 1),
                    )
                nc.vector.tensor_tensor(oh[:, h, :], pv[:, :VD], bcs[:],
                                        op=MULT)
            for nn in range(HID // 512):
                pw = psVW.tile([128, 512], f32, name="pvw")
                for h in range(HPG):
                    nc.tensor.matmul(pw[:], oh[:, h, :],
                                     wot_t[:, h, nn * 512:(nn + 1) * 512],
                                     start=(h == 0), stop=(h == HPG - 1))
                stg = st_p.tile([128, 512], f32, name="stg")
                nc.scalar.copy(stg[:], pw[:])
                nc.sync.dma_start(
                    out=t["out"][qt * 128:(qt + 1) * 128,
                                 nn * 512:(nn + 1) * 512],
                    in_=stg[:])

    dram = ctx.enter_context(tc.tile_pool(name="spill", bufs=1,
                                          space="DRAM"))
    qs_sp = dram.tile([NCH, 128, QBT, CH], bf, name="qs_sp")
    rp_sp = dram.tile([NCH, 128, 2, CH], bf, name="rp_sp")
    rqs_all = []
    for c in range(NCH):
        qs, rp, rqs = phases_abc(c)
        rqs_all.append(rqs)
        nc.sync.dma_start(out=qs_sp[c], in_=qs[:])
        nc.sync.dma_start(out=rp_sp[c], in_=rp[:])
    for c in range(NCH):
        qs = qs_p.tile([128, QBT, CH], bf, name="qs")
        nc.sync.dma_start(out=qs[:], in_=qs_sp[c])
        rp = rope_p.tile([128, 2, CH], bf, name="roped")
        nc.sync.dma_start(out=rp[:], in_=rp_sp[c])
        attention(c, qs, rp, rqs_all[c])


def _build_program(causal):
    import concourse.bass as bass
    import concourse.tile as tile
    from concourse import bacc, mybir

    dt = mybir.dt
    nc = bacc.Bacc("TRN2", target_bir_lowering=False, debug=False,
                   enable_asserts=False, num_devices=NCORES)

    def din(name, shape, dtype=dt.bfloat16):
        return nc.dram_tensor(name, shape, dtype, kind="ExternalInput").ap()

    if causal:
        t = {
            "xT": din("xT", [NCH, 128, KT, CH]),
            "wqa": din("wqa", [QAST, 128, KT, 128]),
            "wcomb": din("wcomb", [QBT, 128, KT, 128]),
            "wkva": din("wkva", [128, KT, 128 + ROPE]),
            "wemb": din("wemb", [128, KVT, HPG * NOPE]),
            "wunb": din("wunb", [128, KVT, HPG * VD]),
            "wot": din("wot", [128, HPG, HID]),
            "cos": din("cos", [128, L]),
            "sin": din("sin", [128, L]),
            "perm": din("perm", [128, 128]),
            "dup": din("dup", [64, 128]),
            "triT": din("triT", [128, 128], dt.float32),
            "out": nc.dram_tensor("out", [L, HID], dt.bfloat16,
                                  kind="ExternalOutput").ap(),
        }
        emit = _emit_fast
    else:
        t = {
            "xT": din("xT", [NCH, KT, 128, CH]),
            "wqa": din("wqa", [MT, KT, 128, 128]),
            "wkva": din("wkva", [KT, 128, KVL + ROPE]),
            "wqb": din("wqb", [QBT, MT, 128, 128]),
            "wemb": din("wemb", [KVT, 128, HPG * NOPE]),
            "wunb": din("wunb", [KVT, 128, HPG * VD]),
            "wot": din("wot", [HPG, 128, HID]),
            "cos": din("cos", [128, L]),
            "sin": din("sin", [128, L]),
            "perm": din("perm", [128, 128]),
            "dup": din("dup", [64, 128]),
            "ident": din("ident", [128, 128]),
            "ident32": din("ident32", [128, 128], dt.float32),
            "out": nc.dram_tensor("out", [L, HID], dt.float32,
                                  kind="ExternalOutput").ap(),
        }
        emit = _emit_full

    with tile.TileContext(nc) as tc:
        with ExitStack() as ctx:
            emit(ctx, tc, t)
    nc.compile()
    return nc


def _get_program(causal):
    if causal not in _PROG_CACHE:
        _PROG_CACHE[causal] = _build_program(causal)
    return _PROG_CACHE[causal]


def _rope_tables():
    freqs = THETA ** (-np.arange(0, ROPE, 2, dtype=np.float64) / ROPE)
    th = np.arange(L, dtype=np.float64)[None, :] * freqs[:, None]  # [32, L]
    cos64 = np.repeat(np.cos(th), 2, axis=0)                       # [64, L]
    sin64 = np.repeat(np.sin(th), 2, axis=0)
    sin64[0::2] *= -1.0
    cosE = np.vstack([cos64, cos64])
    sinE = np.vstack([sin64, sin64])
    return _bf(cosE), _bf(sinE)


def _perm_matrix():
    p64 = np.zeros((64, 64), np.float32)
    for i in range(32):
        p64[2 * i + 1, 2 * i] = 1.0
        p64[2 * i, 2 * i + 1] = 1.0
    pm = np.zeros((128, 128), np.float32)
    pm[:64, :64] = p64
    pm[64:, 64:] = p64
    return _bf(pm)


def _wqb_reorder(wqb_f, heads):
    """q_b rows reordered: nope rows of the 4 heads first, then the pe rows
    packed as two 128-row head pairs. Returns [768, QL]."""
    rows = [wqb_f[h * QKD: h * QKD + NOPE] for h in heads]
    rows += [
        np.vstack([wqb_f[heads[2 * p] * QKD + NOPE:
                         heads[2 * p] * QKD + QKD],
                   wqb_f[heads[2 * p + 1] * QKD + NOPE:
                         heads[2 * p + 1] * QKD + QKD]])
        for p in range(2)
    ]
    return np.vstack(rows)


def make_in_maps(x, mask, wq_a, q_a_ln_w, wq_b, wkv_a, kv_a_ln_w,
                 w_embed_q, w_unembed, wo):
    x = np.asarray(x, np.float32)
    m = np.asarray(mask).reshape(L, L)
    causal = bool(np.array_equal(m, np.tril(np.ones((L, L), bool))))
    if not causal and not m.all():
        raise NotImplementedError("only causal or all-true masks supported")

    wq_a = np.asarray(wq_a, np.float32)
    q_a_ln_w = np.asarray(q_a_ln_w, np.float32)
    wq_b = np.asarray(wq_b, np.float32)
    wkv_a = np.asarray(wkv_a, np.float32)
    kv_a_ln_w = np.asarray(kv_a_ln_w, np.float32)
    w_embed_q = np.asarray(w_embed_q, np.float32)
    w_unembed = np.asarray(w_unembed, np.float32)
    wo = np.asarray(wo, np.float32)

    # fold scalar factors + LN weights into the adjacent matrices
    wqb_f = wq_b * (SQ * SCALE) * q_a_ln_w[None, :]
    kvf = SKV * kv_a_ln_w
    wemb_f = w_embed_q * kvf[None, :, None]     # [H, KVL, NOPE]
    wunb_f = w_unembed * kvf[None, None, :]     # [H, VD, KVL]

    cosE, sinE = _rope_tables()
    permM = _perm_matrix()
    dup = np.zeros((64, 128), np.float32)
    dup[np.arange(128) % 64, np.arange(128)] = 1.0
    dup = _bf(dup)

    in_maps = []
    if causal:
        triT = np.where(
            np.arange(128)[:, None] <= np.arange(128)[None, :], 0.0, -1e30
        ).astype(np.float32)  # [kpos, q]: mask kpos > q
        xT_c = [
            _bf(x[b].T.reshape(KT, 128, NCH, CH).transpose(2, 1, 0, 3))
            for b in range(B)
        ]
        for c in range(NCORES):
            b = c // GPB
            g = c % GPB
            heads = list(range(g * HPG, (g + 1) * HPG))
            wqb_r = _wqb_reorder(wqb_f, heads)           # [768, QL]
            wcomb = wq_a.T @ wqb_r.T                     # [HID, 768]
            wcomb_c = _bf(
                wcomb.reshape(KT, 128, QBT, 128).transpose(2, 1, 0, 3))
            wqa_c = _bf(
                wq_a[g * QAS:(g + 1) * QAS].T.reshape(KT, 128, QAST, 128)
                .transpose(2, 1, 0, 3))
            wkva_c = _bf(np.concatenate(
                [wkv_a[g * 128:(g + 1) * 128], wkv_a[KVL:]], axis=0
            ).T.reshape(KT, 128, 128 + ROPE).transpose(1, 0, 2))
            wemb_c = _bf(
                wemb_f[heads].transpose(1, 0, 2)
                .reshape(KVT, 128, HPG * NOPE).transpose(1, 0, 2))
            wunb_c = _bf(
                wunb_f[heads].transpose(2, 0, 1)
                .reshape(KVT, 128, HPG * VD).transpose(1, 0, 2))
            wot_c = _bf(
                wo[:, g * HPG * VD:(g + 1) * HPG * VD].T
                .reshape(HPG, VD, HID).transpose(1, 0, 2))
            in_maps.append({
                "xT": xT_c[b], "wqa": wqa_c, "wcomb": wcomb_c,
                "wkva": wkva_c, "wemb": wemb_c, "wunb": wunb_c,
                "wot": wot_c, "cos": cosE, "sin": sinE, "perm": permM,
                "dup": dup, "triT": triT,
            })
        return in_maps, causal

    xT_p = [
        _bf(x[b].T.reshape(KT, 128, NCH, CH).transpose(2, 0, 1, 3))
        for b in range(B)
    ]
    wqa_p = _bf(wq_a.T.reshape(KT, 128, MT, 128).transpose(2, 0, 1, 3))
    wkva_p = _bf(wkv_a.T.reshape(KT, 128, KVL + ROPE))
    ident = _bf(np.eye(128, dtype=np.float32))
    for c in range(NCORES):
        b = c // GPB
        g = c % GPB
        heads = list(range(g * HPG, (g + 1) * HPG))
        wqb_r = _wqb_reorder(wqb_f, heads)
        wqb_c = _bf(wqb_r.T.reshape(MT, 128, QBT, 128).transpose(2, 0, 1, 3))
        wemb_c = _bf(
            wemb_f[heads].transpose(1, 0, 2).reshape(KVT, 128, HPG * NOPE))
        wunb_c = _bf(
            wunb_f[heads].transpose(2, 0, 1).reshape(KVT, 128, HPG * VD))
        wot_c = _bf(
            wo[:, g * HPG * VD:(g + 1) * HPG * VD].T.reshape(HPG, VD, HID))
        in_maps.append({
            "xT": xT_p[b], "wqa": wqa_p, "wkva": wkva_p, "wqb": wqb_c,
            "wemb": wemb_c, "wunb": wunb_c, "wot": wot_c,
            "cos": cosE, "sin": sinE, "perm": permM, "dup": dup,
            "ident": ident, "ident32": np.eye(128, dtype=np.float32),
        })
    return in_maps, causal


def assemble(results):
    out = np.zeros((B, L, HID), np.float32)
    for c in range(NCORES):
        out[c // GPB] += results[c]["out"]
    return out


def kernel(**inputs):
    from concourse.bass_utils import run_bass_kernel_spmd

    in_maps, causal = make_in_maps(**inputs)
    nc = _get_program(causal)
    res = run_bass_kernel_spmd(nc, in_maps, list(range(NCORES)))
    return assemble(res.results)
